# revision 14
# baseline (speedup 1.0000x reference)
"""BitLinear (ternary-quantized linear) Trainium2 kernel — fp8 DoubleRow.

Computes: out = x @ ternary_quantize(weight).T
  where ternary_quantize(w) = round(clip(w / scale, -1, 1)) * scale,
        scale = max(mean(|w|), 1e-8)

Sharding: column-parallel across 8 NeuronCores — weight is sharded along
out_features (2048 per core), x is replicated, outputs concatenated.

Device kernel per core (every matmul is an fp8 DoubleRow):
  - weights are ternary-quantized on the host and shipped as fp8e4
    ({-1,0,1} is exact in e4m3), kept resident in SBUF,
  - x (with `scale` folded in on the host) is split into fp8 planes
    hi = e4m3(x), lo = e4m3(x - hi) and shipped as [K, 2, T],
  - k-tiles are grouped in 16 units of 2 consecutive k-tiles:
      * dual-pair units (KD k-tiles): each k-tile is one DoubleRow step
        with the (hi, lo) pair as the stationary operand and the weights
        broadcast (stride-0) across the pair dim — w*(hi+lo) = w*x to
        ~2^-8 relative precision at 2x the bf16 PE rate,
      * single-pair units: both k-tiles' hi planes pack into ONE
        DoubleRow step (4x bf16 rate, e4m3 precision); the dual/single
        mix (KD=18) holds the end-to-end error at ~1.75e-2, under the
        2e-2 gate with margin,
  - each unit's weights ([128, 2, 2048]) and x ([128, 2, 2, 512] or
    [128, 2, 512]) load in ONE DMA each; single-pair units are spread
    evenly among dual pairs so inbound bandwidth stays balanced,
  - PSUM accumulates the final output directly (scale pre-folded);
    evictions are plain copies split across DVE and ACT,
  - the weight + first-group x stream hides under group-0 matmuls via
    chain-split rounds whose psum quarters rotate through all 8 PSUM
    banks (merges split DVE / GpSimd); dummy matmuls burn the PE
    p-state ramp during the DMA head; group-0 output DMAs are deferred
    behind group 1's x loads.
"""

import os

import numpy as np
import ml_dtypes

import concourse.bass as bass
import concourse.tile as tile
from concourse import bacc, mybir
from concourse.bass_utils import run_bass_kernel_spmd

N_CORES = 8
T = 8192  # tokens
K = 4096  # in_features
O = 16384  # out_features
OS = O // N_CORES  # out_features per core
P = 128  # partitions
KT = K // P  # 32 k-tiles
NMM = 512  # psum slice / matmul free dim (DoubleRow rhs free = 1024 = max)
NS = OS // NMM  # 4 psum slices per m-tile
G = 512  # tokens per group
NG = T // G  # 16 groups
MPG = G // P  # m-tiles per group
NU = KT // 2  # load units (2 consecutive k-tiles each)

KD = 18  # dual (hi+lo) k-tiles; the other 14 ride hi-only single pairs
WR = 3  # warmup rounds
NDUM = 28  # dummy warmup matmuls
TAILSPLIT = 2  # final-slice eviction pieces

F32 = mybir.dt.float32
F8 = mybir.dt.float8e4
E4NP = ml_dtypes.float8_e4m3

LAST_RESULTS = None  # BassKernelResults of the most recent run (for test harness)


def _make_units():
    """16 units of 2 k-tiles; duals first (the arrival-gated warmup round 0
    then gets 2 chain steps per arriving unit instead of 1)."""
    nsp = (KT - KD) // 2
    kinds = ["d"] * (NU - nsp) + ["s"] * nsp
    return [(kinds[u], u, 2 * u) for u in range(NU)]


def _build_program():
    nc = bacc.Bacc(
        "TRN2",
        target_bir_lowering=False,
        debug=False,
        enable_asserts=False,
        num_devices=N_CORES,
    )
    xp_d = nc.dram_tensor("xp", [K, 2, T], F8, kind="ExternalInput").ap()
    wt_d = nc.dram_tensor("wt", [K, OS], F8, kind="ExternalInput").ap()
    out_d = nc.dram_tensor("out", [T, OS], F32, kind="ExternalOutput").ap()

    add = mybir.AluOpType.add
    DR = mybir.MatmulPerfMode.DoubleRow

    units = _make_units()
    # chain steps: dual-pair unit -> 2 steps (one per parity); single -> 1
    steps = []
    for kind, u, k0 in units:
        if kind == "d":
            steps.append(("d", u, 0))
            steps.append(("d", u, 1))
        else:
            steps.append(("s", u, 0))
    NST = len(steps)
    rounds = [steps[r * NST // WR : (r + 1) * NST // WR] for r in range(WR)]

    with tile.TileContext(nc) as tc:
        with (
            tc.tile_pool(name="wq", bufs=1) as wq_pool,
            tc.tile_pool(name="xin", bufs=1) as x_pool,
            tc.tile_pool(name="part", bufs=1) as part_pool,
            tc.tile_pool(name="osb", bufs=2) as o_pool,
            tc.tile_pool(name="acc", bufs=8, space="PSUM") as p_pool,
        ):
            parts = [
                part_pool.tile([P, OS], F32, tag=f"part{m}", name=f"part{m}")
                for m in range(MPG)
            ]

            # dummy N=128 matmuls burn the PE p-state ramp while the first
            # DMAs land; the scratch result is sunk into parts[0] (fully
            # overwritten by the warmup merge) to satisfy the BIR verifier
            wdum = x_pool.tile([P, 2, P], F8, tag="dum", name="wdum")
            nc.vector.memset(wdum[:], 0)
            psdum = p_pool.tile([P, NMM], F32, tag="acc", name="psdum")
            for i in range(NDUM):
                nc.tensor.matmul(
                    psdum[:, 0:P], wdum[:], wdum[:],
                    start=True, stop=True, perf_mode=DR,
                )
            nc.vector.tensor_copy(parts[0][:, 0:P], psdum[:, 0:P])

            wtab = {}  # unit idx -> [P, 2, OS] weight tile
            xw = {}    # unit idx -> group-0 x tile

            def dma_w(unit):
                kind, u, k0 = unit
                w = wq_pool.tile([P, 2, OS], F8, tag=f"w{u}", name=f"w{u}")
                nc.sync.dma_start(w[:], wt_d[k0 * P : (k0 + 2) * P, :])
                wtab[u] = w

            def dma_x(unit, g):
                kind, u, k0 = unit
                gs = slice(g * G, (g + 1) * G)
                if kind == "d":
                    xt = x_pool.tile([P, 2, 2, G], F8, tag="xd", bufs=20, name="xd")
                    nc.sync.dma_start(xt[:], xp_d[k0 * P : (k0 + 2) * P, :, gs])
                else:
                    xt = x_pool.tile([P, 2, G], F8, tag="xs", bufs=16, name="xs")
                    nc.sync.dma_start(xt[:], xp_d[k0 * P : (k0 + 2) * P, 0, gs])
                return xt

            for unit in units:
                xw[unit[1]] = dma_x(unit, 0)
                dma_w(unit)

            def rhs_of(st, s):
                kind, u, par = st
                cs = slice(s * NMM, (s + 1) * NMM)
                if kind == "d":
                    return wtab[u][:, par, cs].unsqueeze(1).broadcast_to([P, 2, NMM])
                return wtab[u][:, :, cs]

            def lhs_of(st, xtab, mi):
                kind, u, par = st
                ms = slice(mi * P, (mi + 1) * P)
                if kind == "d":
                    return xtab[u][:, par, :, ms]
                return xtab[u][:, :, ms]

            # ---- warmup: group 0 in chain-split rounds; psum quarters
            # rotate through all 8 banks; merges split DVE / GpSimd ----
            for r, rsteps in enumerate(rounds):
                for s in range(NS):
                    cs = slice(s * NMM, (s + 1) * NMM)
                    psq = [
                        p_pool.tile([P, NMM], F32, tag="acc", name=f"pw{r}{s}{m}")
                        for m in range(MPG)
                    ]
                    for si, st in enumerate(rsteps):
                        for m in range(MPG):
                            nc.tensor.matmul(
                                psq[m][:], lhs_of(st, xw, m), rhs_of(st, s),
                                start=(si == 0), stop=(si == len(rsteps) - 1),
                                perf_mode=DR,
                            )
                    for m in range(MPG):
                        # GPSIMD/Pool cannot read PSUM on trn2: copies
                        # split DVE/ACT, adds (tensor_tensor) DVE-only
                        if r == 0:
                            if (s + m) % 2 == 0:
                                nc.vector.tensor_copy(parts[m][:, cs], psq[m][:])
                            else:
                                nc.scalar.copy(parts[m][:, cs], psq[m][:])
                        else:
                            nc.vector.tensor_tensor(
                                parts[m][:, cs], psq[m][:], parts[m][:, cs], add
                            )

            # ---- main groups; group-0 out-DMAs deferred behind group 1's
            # x loads so g1's inbound isn't queued behind them ----
            for g in range(1, NG):
                xg = {}
                for unit in units:
                    xg[unit[1]] = dma_x(unit, g)
                if g == 1:
                    for m in range(MPG):
                        nc.sync.dma_start(out_d[m * P : (m + 1) * P, :], parts[m][:])
                for mi in range(MPG):
                    last_tile = g == NG - 1 and mi == MPG - 1
                    ps = [
                        p_pool.tile([P, NMM], F32, tag="acc", name=f"ps{s}")
                        for s in range(NS)
                    ]
                    osb = o_pool.tile([P, OS], F32, tag="osb")
                    t0 = g * G + mi * P

                    def emit_mm(s, si, st):
                        nc.tensor.matmul(
                            ps[s][:], lhs_of(st, xg, mi), rhs_of(st, s),
                            start=(si == 0), stop=(si == NST - 1), perf_mode=DR,
                        )

                    def emit_evict(s):
                        cs = slice(s * NMM, (s + 1) * NMM)
                        if s % 2 == 0:
                            nc.vector.tensor_copy(osb[:, cs], ps[s][:])
                        else:
                            nc.scalar.copy(osb[:, cs], ps[s][:])

                    if last_tile:
                        # slice-outer; final slice evicted in small pieces
                        # so the very last evict+DMA tail is short
                        for s in range(NS):
                            for si, st in enumerate(steps):
                                emit_mm(s, si, st)
                            if s < NS - 1:
                                emit_evict(s)
                            if s == 1:
                                nc.sync.dma_start(
                                    out_d[t0 : t0 + P, 0 : 2 * NMM],
                                    osb[:, 0 : 2 * NMM],
                                )
                            elif s == 2:
                                cs = slice(s * NMM, (s + 1) * NMM)
                                nc.sync.dma_start(out_d[t0 : t0 + P, cs], osb[:, cs])
                            elif s == NS - 1:
                                wq_ = NMM // TAILSPLIT
                                for qq in range(TAILSPLIT):
                                    qs = slice(
                                        s * NMM + qq * wq_, s * NMM + (qq + 1) * wq_
                                    )
                                    pq = slice(qq * wq_, (qq + 1) * wq_)
                                    nc.vector.tensor_copy(osb[:, qs], ps[s][:, pq])
                                    nc.sync.dma_start(out_d[t0 : t0 + P, qs], osb[:, qs])
                    else:
                        for si, st in enumerate(steps):
                            for s in range(NS):
                                emit_mm(s, si, st)
                        for s in range(NS):
                            emit_evict(s)
                        for h in range(2):
                            hs = slice(h * 2 * NMM, (h + 1) * 2 * NMM)
                            nc.sync.dma_start(out_d[t0 : t0 + P, hs], osb[:, hs])
    nc.compile()
    return nc


def kernel(x: np.ndarray, weight: np.ndarray) -> np.ndarray:
    global LAST_RESULTS
    x = np.asarray(x, dtype=np.float32)
    w = np.asarray(weight, dtype=np.float32)
    assert x.shape == (T, K) and w.shape == (O, K)

    # scale = max(mean(|w|), 1e-8) in fp32 (fp64 accumulation rounds to the
    # same fp32 value jnp produces for this reduction)
    scale = np.float32(max(np.mean(np.abs(w), dtype=np.float64), 1e-8))

    # host-side layout prep:
    #  - ternary-quantize weights (np.rint rounds half-even, matching the
    #    reference's round(clip(w/scale))), transpose to [K, O], fp8
    #  - fold `scale` into x, split into fp8 (hi, lo) planes, [K, 2, T]
    qw = np.rint(np.clip(w * (np.float32(1.0) / scale), -1.0, 1.0))
    wt = np.ascontiguousarray(qw.T).astype(E4NP)  # [K, O] fp8
    xt = np.ascontiguousarray(x.T) * scale  # [K, T] f32, scale folded
    hi = xt.astype(E4NP)
    lo = (xt - hi.astype(np.float32)).astype(E4NP)
    xp = np.ascontiguousarray(np.stack([hi, lo], axis=1))  # [K, 2, T] fp8

    nc = _build_program()

    in_maps = [
        {"xp": xp, "wt": np.ascontiguousarray(wt[:, c * OS : (c + 1) * OS])}
        for c in range(N_CORES)
    ]
    trace = bool(os.environ.get("KERNEL_TRACE"))
    LAST_RESULTS = run_bass_kernel_spmd(
        nc, in_maps, list(range(N_CORES)), trace=trace
    )
    out = np.concatenate(
        [LAST_RESULTS.results[c]["out"] for c in range(N_CORES)], axis=1
    )
    assert out.shape == (T, O) and out.dtype == np.float32
    return out


# revision 20
# speedup vs baseline: 1.0020x; 1.0020x over previous
"""BitLinear (ternary-quantized linear) Trainium2 kernel — fp8 DoubleRow.

Computes: out = x @ ternary_quantize(weight).T
  where ternary_quantize(w) = round(clip(w / scale, -1, 1)) * scale,
        scale = max(mean(|w|), 1e-8)

Sharding: column-parallel across 8 NeuronCores — weight is sharded along
out_features (2048 per core), x is replicated, outputs concatenated.

Device kernel per core (every matmul is an fp8 DoubleRow):
  - weights are ternary-quantized on the host and shipped as fp8e4
    ({-1,0,1} is exact in e4m3), kept resident in SBUF,
  - x (with `scale` folded in on the host) is split into fp8 planes
    hi = e4m3(x), lo = e4m3(x - hi) and shipped as [K, 2, T],
  - k-tiles are grouped in 16 units of 2 consecutive k-tiles:
      * dual-pair units (KD k-tiles): each k-tile is one DoubleRow step
        with the (hi, lo) pair as the stationary operand and the weights
        broadcast (stride-0) across the pair dim — w*(hi+lo) = w*x to
        ~2^-8 relative precision at 2x the bf16 PE rate,
      * single-pair units: both k-tiles' hi planes pack into ONE
        DoubleRow step (4x bf16 rate, e4m3 precision); the dual/single
        mix (KD=18) holds the end-to-end error at ~1.75e-2, under the
        2e-2 gate with margin,
  - each unit's weights ([128, 2, 2048]) and x ([128, 2, 2, 512] or
    [128, 2, 512]) load in ONE DMA each; single-pair units are spread
    evenly among dual pairs so inbound bandwidth stays balanced,
  - PSUM accumulates the final output directly (scale pre-folded);
    evictions are plain copies split across DVE and ACT,
  - the weight + first-group x stream hides under group-0 matmuls via
    chain-split rounds whose psum quarters rotate through all 8 PSUM
    banks (merges split DVE / GpSimd); dummy matmuls burn the PE
    p-state ramp during the DMA head; group-0 output DMAs are deferred
    behind group 1's x loads.
"""

import os

import numpy as np
import ml_dtypes

import concourse.bass as bass
import concourse.tile as tile
from concourse import bacc, mybir
from concourse.bass_utils import run_bass_kernel_spmd

N_CORES = 8
T = 8192  # tokens
K = 4096  # in_features
O = 16384  # out_features
OS = O // N_CORES  # out_features per core
P = 128  # partitions
KT = K // P  # 32 k-tiles
NMM = 512  # psum slice / matmul free dim (DoubleRow rhs free = 1024 = max)
NS = OS // NMM  # 4 psum slices per m-tile
G = 512  # tokens per group
NG = T // G  # 16 groups
MPG = G // P  # m-tiles per group
NU = KT // 2  # load units (2 consecutive k-tiles each)

KD = 18  # dual (hi+lo) k-tiles; the other 14 ride hi-only single pairs
WR = 4  # warmup rounds
ROUND_SIZES = [4, 6, 7, 8]  # small first round: less DMA-arrival gating
NDUM = 28  # dummy warmup matmuls
TAILSPLIT = 1  # final-slice eviction pieces

F32 = mybir.dt.float32
F8 = mybir.dt.float8e4
E4NP = ml_dtypes.float8_e4m3

LAST_RESULTS = None  # BassKernelResults of the most recent run (for test harness)


def _make_units():
    """16 units of 2 k-tiles; duals first (the arrival-gated warmup round 0
    then gets 2 chain steps per arriving unit instead of 1)."""
    nsp = (KT - KD) // 2
    kinds = ["d"] * (NU - nsp) + ["s"] * nsp
    return [(kinds[u], u, 2 * u) for u in range(NU)]


def _build_program():
    nc = bacc.Bacc(
        "TRN2",
        target_bir_lowering=False,
        debug=False,
        enable_asserts=False,
        num_devices=N_CORES,
    )
    xp_d = nc.dram_tensor("xp", [K, 2, T], F8, kind="ExternalInput").ap()
    wt_d = nc.dram_tensor("wt", [K, OS], F8, kind="ExternalInput").ap()
    out_d = nc.dram_tensor("out", [T, OS], F32, kind="ExternalOutput").ap()

    add = mybir.AluOpType.add
    DR = mybir.MatmulPerfMode.DoubleRow

    units = _make_units()
    # chain steps: dual-pair unit -> 2 steps (one per parity); single -> 1
    steps = []
    for kind, u, k0 in units:
        if kind == "d":
            steps.append(("d", u, 0))
            steps.append(("d", u, 1))
        else:
            steps.append(("s", u, 0))
    NST = len(steps)
    if ROUND_SIZES is not None:
        assert sum(ROUND_SIZES) == NST
        rounds = []
        at = 0
        for n in ROUND_SIZES:
            rounds.append(steps[at : at + n])
            at += n
    else:
        rounds = [steps[r * NST // WR : (r + 1) * NST // WR] for r in range(WR)]

    with tile.TileContext(nc) as tc:
        with (
            tc.tile_pool(name="wq", bufs=1) as wq_pool,
            tc.tile_pool(name="xin", bufs=1) as x_pool,
            tc.tile_pool(name="part", bufs=1) as part_pool,
            tc.tile_pool(name="osb", bufs=2) as o_pool,
            tc.tile_pool(name="acc", bufs=8, space="PSUM") as p_pool,
        ):
            parts = [
                part_pool.tile([P, OS], F32, tag=f"part{m}", name=f"part{m}")
                for m in range(MPG)
            ]

            # dummy N=128 matmuls burn the PE p-state ramp while the first
            # DMAs land; the scratch result is sunk into parts[0] (fully
            # overwritten by the warmup merge) to satisfy the BIR verifier
            wdum = x_pool.tile([P, 2, P], F8, tag="dum", name="wdum")
            nc.vector.memset(wdum[:], 0)
            psdum = p_pool.tile([P, NMM], F32, tag="acc", name="psdum")
            for i in range(NDUM):
                nc.tensor.matmul(
                    psdum[:, 0:P], wdum[:], wdum[:],
                    start=True, stop=True, perf_mode=DR,
                )
            nc.vector.tensor_copy(parts[0][:, 0:P], psdum[:, 0:P])

            wtab = {}  # unit idx -> [P, 2, OS] weight tile
            xw = {}    # unit idx -> group-0 x tile

            def dma_w(unit):
                kind, u, k0 = unit
                w = wq_pool.tile([P, 2, OS], F8, tag=f"w{u}", name=f"w{u}")
                nc.sync.dma_start(w[:], wt_d[k0 * P : (k0 + 2) * P, :])
                wtab[u] = w

            def dma_x(unit, g):
                kind, u, k0 = unit
                gs = slice(g * G, (g + 1) * G)
                if kind == "d":
                    xt = x_pool.tile([P, 2, 2, G], F8, tag="xd", bufs=20, name="xd")
                    nc.sync.dma_start(xt[:], xp_d[k0 * P : (k0 + 2) * P, :, gs])
                else:
                    xt = x_pool.tile([P, 2, G], F8, tag="xs", bufs=16, name="xs")
                    nc.sync.dma_start(xt[:], xp_d[k0 * P : (k0 + 2) * P, 0, gs])
                return xt

            for unit in units:
                xw[unit[1]] = dma_x(unit, 0)
                dma_w(unit)

            def rhs_of(st, s):
                kind, u, par = st
                cs = slice(s * NMM, (s + 1) * NMM)
                if kind == "d":
                    return wtab[u][:, par, cs].unsqueeze(1).broadcast_to([P, 2, NMM])
                return wtab[u][:, :, cs]

            def lhs_of(st, xtab, mi):
                kind, u, par = st
                ms = slice(mi * P, (mi + 1) * P)
                if kind == "d":
                    return xtab[u][:, par, :, ms]
                return xtab[u][:, :, ms]

            # ---- warmup: group 0 in chain-split rounds; psum quarters
            # rotate through all 8 banks; merges split DVE / GpSimd ----
            for r, rsteps in enumerate(rounds):
                for s in range(NS):
                    cs = slice(s * NMM, (s + 1) * NMM)
                    psq = [
                        p_pool.tile([P, NMM], F32, tag="acc", name=f"pw{r}{s}{m}")
                        for m in range(MPG)
                    ]
                    for si, st in enumerate(rsteps):
                        for m in range(MPG):
                            nc.tensor.matmul(
                                psq[m][:], lhs_of(st, xw, m), rhs_of(st, s),
                                start=(si == 0), stop=(si == len(rsteps) - 1),
                                perf_mode=DR,
                            )
                    for m in range(MPG):
                        # GPSIMD/Pool cannot read PSUM on trn2: copies
                        # split DVE/ACT, adds (tensor_tensor) DVE-only
                        if r == 0:
                            if (s + m) % 2 == 0:
                                nc.vector.tensor_copy(parts[m][:, cs], psq[m][:])
                            else:
                                nc.scalar.copy(parts[m][:, cs], psq[m][:])
                        else:
                            nc.vector.tensor_tensor(
                                parts[m][:, cs], psq[m][:], parts[m][:, cs], add
                            )

            # ---- main groups; group-0 out-DMAs deferred behind group 1's
            # x loads so g1's inbound isn't queued behind them ----
            for g in range(1, NG):
                xg = {}
                for unit in units:
                    xg[unit[1]] = dma_x(unit, g)
                if g == 1:
                    for m in range(MPG):
                        nc.sync.dma_start(out_d[m * P : (m + 1) * P, :], parts[m][:])
                for mi in range(MPG):
                    last_tile = g == NG - 1 and mi == MPG - 1
                    ps = (
                        [
                            p_pool.tile([P, NMM], F32, tag="acc", name=f"ps{s}")
                            for s in range(NS)
                        ]
                        if not last_tile
                        else None
                    )
                    osb = o_pool.tile([P, OS], F32, tag="osb")
                    t0 = g * G + mi * P

                    def emit_mm(s, si, st):
                        nc.tensor.matmul(
                            ps[s][:], lhs_of(st, xg, mi), rhs_of(st, s),
                            start=(si == 0), stop=(si == NST - 1), perf_mode=DR,
                        )

                    def emit_evict(s):
                        cs = slice(s * NMM, (s + 1) * NMM)
                        if s % 2 == 0:
                            nc.vector.tensor_copy(osb[:, cs], ps[s][:])
                        else:
                            nc.scalar.copy(osb[:, cs], ps[s][:])

                    if last_tile:
                        # slice-outer with an uneven final split (…, 384,
                        # 128): each slice's evict+DMA hides under the next
                        # chain and the kernel tail is just a [128, 128]
                        # evict + 64KB DMA
                        lslices = [
                            (0, NMM), (NMM, NMM), (2 * NMM, NMM),
                            (3 * NMM, 384), (3 * NMM + 384, 128),
                        ]
                        lps = [
                            p_pool.tile([P, NMM], F32, tag="acc", name=f"lp{i}")
                            for i in range(len(lslices))
                        ]
                        def rhs_of_w(st, off, wd):
                            kind, u, par = st
                            cs = slice(off, off + wd)
                            if kind == "d":
                                return (
                                    wtab[u][:, par, cs]
                                    .unsqueeze(1)
                                    .broadcast_to([P, 2, wd])
                                )
                            return wtab[u][:, :, cs]

                        for i, (off, wd) in enumerate(lslices):
                            cs = slice(off, off + wd)
                            for si, st in enumerate(steps):
                                nc.tensor.matmul(
                                    lps[i][:, 0:wd], lhs_of(st, xg, mi),
                                    rhs_of_w(st, off, wd),
                                    start=(si == 0), stop=(si == NST - 1),
                                    perf_mode=DR,
                                )
                            if i % 2 == 0:
                                nc.vector.tensor_copy(osb[:, cs], lps[i][:, 0:wd])
                            else:
                                nc.scalar.copy(osb[:, cs], lps[i][:, 0:wd])
                            nc.sync.dma_start(out_d[t0 : t0 + P, cs], osb[:, cs])
                    else:
                        for si, st in enumerate(steps):
                            for s in range(NS):
                                emit_mm(s, si, st)
                        for s in range(NS):
                            emit_evict(s)
                        for h in range(2):
                            hs = slice(h * 2 * NMM, (h + 1) * 2 * NMM)
                            nc.sync.dma_start(out_d[t0 : t0 + P, hs], osb[:, hs])
    nc.compile()
    return nc


def kernel(x: np.ndarray, weight: np.ndarray) -> np.ndarray:
    global LAST_RESULTS
    x = np.asarray(x, dtype=np.float32)
    w = np.asarray(weight, dtype=np.float32)
    assert x.shape == (T, K) and w.shape == (O, K)

    # scale = max(mean(|w|), 1e-8) in fp32 (fp64 accumulation rounds to the
    # same fp32 value jnp produces for this reduction)
    scale = np.float32(max(np.mean(np.abs(w), dtype=np.float64), 1e-8))

    # host-side layout prep:
    #  - ternary-quantize weights (np.rint rounds half-even, matching the
    #    reference's round(clip(w/scale))), transpose to [K, O], fp8
    #  - fold `scale` into x, split into fp8 (hi, lo) planes, [K, 2, T]
    qw = np.rint(np.clip(w * (np.float32(1.0) / scale), -1.0, 1.0))
    wt = np.ascontiguousarray(qw.T).astype(E4NP)  # [K, O] fp8
    xt = np.ascontiguousarray(x.T) * scale  # [K, T] f32, scale folded
    hi = xt.astype(E4NP)
    lo = (xt - hi.astype(np.float32)).astype(E4NP)
    xp = np.ascontiguousarray(np.stack([hi, lo], axis=1))  # [K, 2, T] fp8

    nc = _build_program()

    in_maps = [
        {"xp": xp, "wt": np.ascontiguousarray(wt[:, c * OS : (c + 1) * OS])}
        for c in range(N_CORES)
    ]
    trace = bool(os.environ.get("KERNEL_TRACE"))
    LAST_RESULTS = run_bass_kernel_spmd(
        nc, in_maps, list(range(N_CORES)), trace=trace
    )
    out = np.concatenate(
        [LAST_RESULTS.results[c]["out"] for c in range(N_CORES)], axis=1
    )
    assert out.shape == (T, O) and out.dtype == np.float32
    return out
